# revision 52
# baseline (speedup 1.0000x reference)
"""Trainium2 Bass kernel for nn_FCClassifier (predictive-coding FC network).

Data-parallel over batch (1024 -> 128 rows/core on 8 cores); state in SBUF as
[128, width] fp32 (x) / fp8 (16*e).  Per settling step the sweeps are MERGED
per layer, ordered so the HBM weight stream never pauses:
  fwd3 | fwd4+bwd4 (SBUF-resident W4) | bwd3 | fwd2 | bwd2 | g1
with, per layer:
  fwd li:  pred_{li-1} = tanh(x_li) @ W_li^T   (fp8 DoubleRow, fp32 PSUM)
           e_{li-1} = x_{li-1} - pred + noise_eff   (fp8, clamped +-224)
  bwd li:  g_li = e_{li-1} @ W_li ; x_li += 0.1*(g*(1-tanh(x_li)^2) - e_li)
Every operand read is the pre-update value, so the merged order is EXACT wrt
the reference's top-down-then-bottom-up schedule.

Layer 1 exploits the pinned observation:  g1 = e0 @ W1 with
e0 = x0 - t1 W1^T - b0 + nu0  =>  g1 = B(s) - t1 (W1^T W1), where
B(s) = 256*((x0 + nu0(s) - b0) @ W1) is host-precomputed per step and
streamed bf16 (1MB/step).  This removes fwd1+bwd1 (25.2MB/step) for
G1 = W1^T W1 (16.8MB/step) and kills e0/x0 entirely.

Weight streaming: one fp8 e4m3 blob (x16, clipped +-240; G1 x256) packed as
DoubleRow k-tile PAIRS in DMA-segment rows -- each row is one DMA payload,
p-major, 16KB contiguous per partition.  W4 (both orientations) plus the
leading fwd3 seg are SBUF-resident.  Stationary operands (tanh^T, e^T fp8)
are built with bf16 PE transposes + clamping fp8 copy-out, incrementally as
each e-chunk closes.  d_actv factors are precomputed per chunk BEFORE a
group's matmuls close so one DVE op frees each PSUM bank (no PE stall on
pool rotation).  tT3/tT4 are rebuilt a step ahead (right after bwd3/bwd4)
so the PE rolls across the loop boundary hot; dummy transposes after each
For_i all-engine barrier keep the HAM clock at 2.4GHz.  Steps run in a
hardware For_i loop, UNROLL steps per iteration.

The INIT pass (x_li = x_{li-1} @ W_li) runs in f32r from a bf16 blob: the
settling dynamics are insensitive to loop quantization but the output tracks
init errors ~1:1.  The bf16->f32r upcast alternates DVE/ACT and stages in
the (settle-idle) weight-stream pool; init-only pools are scoped and
released before the loop so the resident weights reuse their SBUF.
"""
import contextlib
import numpy as np
import ml_dtypes

import concourse.bass as bass
import concourse.tile as tile
from concourse import bacc, mybir
from concourse import bass_utils

SIZES = [3072, 4096, 4096, 2048, 1000]
BATCH = 1024
GAMMA = 0.1
NOISE_SCALE = 0.034
N_CORES = 8
ROWS = BATCH // N_CORES  # 128

BF16 = mybir.dt.bfloat16
F8 = mybir.dt.float8e4
F32 = mybir.dt.float32
F32R = mybir.dt.float32r
NPBF16 = ml_dtypes.bfloat16
NPF8 = ml_dtypes.float8_e4m3fn

WS = 16.0    # weight scale (and e scale); fwd acc = 16*pred, bwd acc = 256*g
NGROUP = 4   # accumulator banks per matmul group
CHUNK = 512  # psum chunk / vector chunk
SEGP = 4     # k-tile PAIRS per fp8 weight-stream DMA segment
KSEG = 1     # k-tiles per f32r init weight-stream DMA segment

W_TOT = sum(SIZES[1:])    # 11240 (x for layers 1..4; x0 never read since
                          # the G-trick absorbed e0 into the host B-stream)
E_TOT = sum(SIZES[1:4])   # 10240 (e/noise for layers 1..3; e0 eliminated)
XOFS = {1: 0, 2: SIZES[1], 3: SIZES[1] + SIZES[2],
        4: SIZES[1] + SIZES[2] + SIZES[3]}
EOFS = {1: 0, 2: SIZES[1], 3: SIZES[1] + SIZES[2]}  # e offsets, lo=1..3

DR = mybir.MatmulPerfMode.DoubleRow

SEG_ELEMS = SEGP * NGROUP * 2 * CHUNK   # 8192 fp8 elems per full seg row
SEG_ELEMS_I = KSEG * NGROUP * CHUNK     # 2048 f32 elems per full init row

# merged per-step order; layer 4 (both orientations) is SBUF-resident.
# fwd3 leads so the weight stream restarts immediately after the loop
# boundary; the resident layer-4 phase overlaps bwd3/fwd2 prefetch.
#
# Layer 1 uses the pinned-x0 identity:  g1 = e0 @ W1 with
# e0 = x0 - tanh(x1) W1^T - b0 + nu0  =>  g1 = B(s) - tanh(x1) (W1^T W1)
# where B(s) = (x0 + nu0(s) - b0) @ W1 is host-precomputed per step (x0 is
# the pinned observation).  This removes both the fwd1 and bwd1 weight
# streams (25.2MB/step) in exchange for G1 = W1^T W1 (16.8MB/step) and a
# 1MB/step B stream, and removes e0/eT0 entirely.
STREAM_ENTRIES = [("fwd", 3), ("bwd", 3), ("fwd", 2), ("bwd", 2),
                  ("g1", 1)]
RES_ENTRIES = [("fwd", 4), ("bwd", 4)]
UNROLL = 10  # settle steps per For_i iteration (fewer all-engine barriers)
GS = 256.0   # G1 weight scale (g1 acc = 256*t1G1); B stream is 256*B
N_PIN = 2    # leading stream segs (fwd3 head) pinned resident in SBUF


def _cdiv(a, b):
    return (a + b - 1) // b


def _chunks(total, size):
    return [(o, min(size, total - o)) for o in range(0, total, size)]


def _groups(lst, n):
    return [lst[i:i + n] for i in range(0, len(lst), n)]


def _sweep_dims(kind, li):
    if kind == "fwd":
        return SIZES[li], SIZES[li - 1]   # K, N
    if kind == "g1":
        return SIZES[1], SIZES[1]         # t1 @ (W1^T W1)
    return SIZES[li - 1], SIZES[li]


def _seg_iter_one(kind, li):
    """Yield (grp, seg0, seg_n, kp_n) for one layer of one sweep."""
    K, N = _sweep_dims(kind, li)
    kt_n = _cdiv(K, 128)
    kp_n = _cdiv(kt_n, 2) if kind != "init" else kt_n
    per = SEGP if kind != "init" else KSEG
    for grp in _groups(_chunks(N, CHUNK), NGROUP):
        for seg0 in range(0, kp_n, per):
            yield grp, seg0, min(per, kp_n - seg0), kp_n


def _entry_segs(kind, li):
    return sum(1 for _ in _seg_iter_one(kind, li))


def _seg_used(kind, li, grp, seg_n):
    if kind == "init":
        return seg_n * len(grp) * CHUNK
    return seg_n * len(grp) * 2 * CHUNK


N_SEGS_S = sum(_entry_segs(k, li) for k, li in STREAM_ENTRIES)   # 76
N_SEGS_I = sum(_entry_segs("init", li) for li in (1, 2, 3, 4))   # 80
RES_SEGS = {e: _entry_segs(*e) for e in RES_ENTRIES}             # fwd4:2 bwd4:4
RES_USED = {e: [ _seg_used(e[0], e[1], grp, seg_n)
                 for grp, seg0, seg_n, _ in _seg_iter_one(*e)]
            for e in RES_ENTRIES}


# ---------------------------------------------------------------- host prep

def _noise_eff(steps, bs, obs, W1):
    """noise fp8 [steps,1024,E_TOT] (x16, layers 1..3) and the layer-1
    B-stream bf16 [steps,1024,SIZES[1]]: 256*((x0 + nu0 - b0) @ W1)."""
    import jax, jax.numpy as jnp
    cpu = jax.devices("cpu")[0]
    with jax.default_device(cpu):
        nkey = jax.random.key(42)
        rows, brows = [], []
        for i in range(steps):
            temp = np.float32(1.0 - np.float32(i) / steps)
            pieces = []
            for lo in range(4):
                k = jax.random.fold_in(jax.random.fold_in(nkey, i), lo)
                nz = np.asarray(jax.random.normal(k, (BATCH, SIZES[lo]), jnp.float32))
                nz = nz * np.float32(NOISE_SCALE) * temp - bs[lo][None, :]
                if lo == 0:
                    brows.append(((obs + nz) @ W1) * np.float32(GS))
                else:
                    pieces.append(nz * np.float32(WS))
            rows.append(np.concatenate(pieces, axis=1))
        noise = np.clip(np.stack(rows), -240.0, 240.0).astype(NPF8)
        bstream = np.stack(brows).astype(NPBF16)
        return noise, bstream


def _pack_rows_fp8(Ws, entries):
    """fp8 blob rows [n_segs, 128*SEG_ELEMS] for (kind, li) entries in order.

    Seg row layout per partition p: for si (pair), for gi (chunk): the
    DoubleRow pair-block [2, CHUNK] flattened; i.e. elem offset
    (si*NGROUP_eff+gi)*2*CHUNK + i*CHUNK + n."""
    rows = []
    for kind, li in entries:
        if kind == "fwd":
            Wm, scale = Ws[li - 1].T, WS
        elif kind == "g1":
            W1 = Ws[0]
            Wm, scale = (W1.T @ W1).astype(np.float32), GS
        else:
            Wm, scale = Ws[li - 1], WS
        K, N = Wm.shape
        for grp, seg0, seg_n, kp_n in _seg_iter_one(kind, li):
            row = np.zeros((128, SEG_ELEMS), np.float32)
            for si in range(seg_n):
                kp = seg0 + si
                for gi, (n0, nw) in enumerate(grp):
                    base = (si * len(grp) + gi) * 2 * CHUNK
                    for i in (0, 1):
                        k0 = (2 * kp + i) * 128
                        kw = min(128, K - k0)
                        if kw > 0:
                            row[:kw, base + i * CHUNK: base + i * CHUNK + nw] = \
                                Wm[k0:k0 + kw, n0:n0 + nw]
            rows.append((row * np.float32(scale)).reshape(1, -1))
    out = np.concatenate(rows, 0)
    return np.clip(out, -240.0, 240.0).astype(NPF8)


def _pack_init(Ws):
    """bf16 init blob; upcast on-chip to f32r for init matmuls."""
    rows = []
    for li in (1, 2, 3, 4):
        Wm = Ws[li - 1]
        K, N = Wm.shape
        for grp, seg0, seg_n, kt_n in _seg_iter_one("init", li):
            row = np.zeros((128, SEG_ELEMS_I), np.float32)
            for si in range(seg_n):
                kt = seg0 + si
                k0, kw = kt * 128, min(128, K - kt * 128)
                for gi, (n0, nw) in enumerate(grp):
                    base = (si * len(grp) + gi) * CHUNK
                    row[:kw, base:base + nw] = Wm[k0:k0 + kw, n0:n0 + nw]
            rows.append(row.reshape(1, -1))
    return np.concatenate(rows, 0).astype(NPBF16)


# ---------------------------------------------------------------- builder

class _C:
    pass


def _seg_dma(c, pool, blob, seg_idx, used, dtype, row_elems):
    wt = pool.tile([128, row_elems], dtype, tag="wseg", name="wt")
    nc = c.nc
    nc.sync.dma_start(
        wt[:, :used],
        blob[seg_idx:seg_idx + 1]
            .rearrange("b (p f) -> p (b f)", p=128)[:, :used])
    return wt


def _emit_layer_mm(c, li, kind, lhsT, out_cb, blob=None, ofs=None, res=None,
                   pre_cb=None, res_prefix=()):
    """DoubleRow fp8 matmuls for one layer of a sweep.

    Streams seg rows from `blob` (advancing ofs) or consumes resident
    SBUF tiles from `res` (list indexed by seg order).  `res_prefix`
    supplies resident tiles for the first len() segs of a streamed
    layer (ofs must already skip those blob rows).  `pre_cb(li, n0,
    nw)` is emitted per chunk before the group's matmuls so x-only work
    (d_actv) overlaps the stream instead of bursting at group close."""
    nc = c.nc
    K, N = _sweep_dims(kind, li)
    kp_n = _cdiv(_cdiv(K, 128), 2)
    seg_i = 0
    for grp in _groups(_chunks(N, CHUNK), NGROUP):
        if pre_cb is not None:
            for (n0, nw) in grp:
                pre_cb(li, n0, nw)
        accs = [c.apool.tile([128, CHUNK], F32, tag="acc", name="acc") for _ in grp]
        for seg0 in range(0, kp_n, SEGP):
            seg_n = min(SEGP, kp_n - seg0)
            nblk = seg_n * len(grp)
            if res is not None:
                wt = res[seg_i]
            elif seg_i < len(res_prefix):
                wt = res_prefix[seg_i]
            else:
                wt = _seg_dma(c, c.wpool, blob, ofs[0], nblk * 2 * CHUNK, F8,
                              SEG_ELEMS)
                ofs[0] += 1
            seg_i += 1
            for si in range(seg_n):
                kp = seg0 + si
                for gi, (n0, nw) in enumerate(grp):
                    bi = si * len(grp) + gi
                    nc.tensor.matmul(
                        accs[gi][:, :nw],
                        lhsT[:, kp * 256:(kp + 1) * 256]
                            .rearrange("p (two m) -> p two m", two=2),
                        wt[:, bi * 2 * CHUNK:(bi + 1) * 2 * CHUNK]
                            .rearrange("p (two f) -> p two f", two=2)[:, :, :nw],
                        start=(kp == 0), stop=(kp == kp_n - 1),
                        perf_mode=DR)
        for gi, (n0, nw) in enumerate(grp):
            out_cb(li, n0, nw, accs[gi])


def _emit_layer_mm_init(c, li, lhsT, blob, ofs, out_cb):
    """f32r normal-mode matmuls for one init layer (seg-row streaming).

    The bf16->f32r upcast alternates between DVE and ACT so neither
    engine serializes the init pipeline."""
    nc = c.nc
    K, N = _sweep_dims("init", li)
    kt_n = _cdiv(K, 128)
    for grp in _groups(_chunks(N, CHUNK), NGROUP):
        accs = [c.apool.tile([128, CHUNK], F32, tag="acc", name="acc") for _ in grp]
        for seg0 in range(0, kt_n, KSEG):
            seg_n = min(KSEG, kt_n - seg0)
            nblk = seg_n * len(grp)
            wt8 = _seg_dma(c, c.ipool, blob, ofs[0], nblk * CHUNK, BF16, SEG_ELEMS_I)
            # stage in the (settle-idle) weight-stream pool: same byte size
            # per tile, so the 7-buf rotation double-buffers the upcast
            wt = c.wpool.tile([128, SEG_ELEMS // 4], F32R, tag="wseg",
                              name="wstage")
            if ofs[0] % 2 == 0:
                nc.vector.tensor_copy(wt[:, :nblk * CHUNK], wt8[:, :nblk * CHUNK])
            else:
                nc.scalar.activation(wt[:, :nblk * CHUNK], wt8[:, :nblk * CHUNK],
                                     mybir.ActivationFunctionType.Copy)
            ofs[0] += 1
            for si in range(seg_n):
                kt = seg0 + si
                kw = min(128, K - kt * 128)
                for gi, (n0, nw) in enumerate(grp):
                    bi = si * len(grp) + gi
                    nc.tensor.matmul(
                        accs[gi][:, :nw],
                        lhsT[:kw, kt * 128:kt * 128 + 128],
                        wt[:kw, bi * CHUNK:bi * CHUNK + nw],
                        start=(kt == 0), stop=(kt == kt_n - 1))
        for gi, (n0, nw) in enumerate(grp):
            out_cb(li, n0, nw, accs[gi])


def _emit_transpose(c, dst, src, width, clamp=True, pdt=BF16):
    """PE-transpose [128, width] src -> dst [128, kt_n*128].

    fp8 dst copy-out clamps to +-224 so outliers can't hit the fp8 Inf."""
    nc = c.nc

    def _cpy(d, s):
        if clamp:
            nc.vector.tensor_scalar(d, s, 224.0, -224.0,
                                    mybir.AluOpType.min, mybir.AluOpType.max)
        else:
            nc.vector.tensor_copy(d, s)

    grp_n = 8 if pdt == BF16 else 4     # keep pt at <=2KB/partition either way
    ident = c.ident if pdt == BF16 else c.ident32
    kt_n = _cdiv(width, 128)
    for base in range(0, kt_n, grp_n):
        nt = min(grp_n, kt_n - base)
        pt = c.tpool.tile([128, grp_n * 128], pdt, tag="tr")
        kws = []
        for j in range(nt):
            kt = base + j
            kw = min(128, width - kt * 128)
            kws.append(kw)
            nc.tensor.transpose(
                pt[:kw, j * 128:j * 128 + 128],
                src[:, kt * 128:kt * 128 + kw],
                ident[:, :])
        if all(k == 128 for k in kws):
            _cpy(dst[:, base * 128:(base + nt) * 128], pt[:, :nt * 128])
        else:
            nfull = sum(1 for k in kws if k == 128)
            if nfull:
                _cpy(dst[:, base * 128:(base + nfull) * 128],
                     pt[:, :nfull * 128])
            for j in range(nfull, nt):
                kw = kws[j]
                _cpy(dst[:kw, (base + j) * 128:(base + j) * 128 + 128],
                     pt[:kw, j * 128:j * 128 + 128])


def build(steps):
    nc = bacc.Bacc("TRN2", target_bir_lowering=False, debug=False,
                   num_devices=N_CORES)
    c = _C()
    c.nc = nc

    obsT_d = nc.dram_tensor("obsT", [SIZES[0] // 128, 128 * ROWS], F32R,
                            kind="ExternalInput").ap()
    ws_d = nc.dram_tensor("ws_blob", [N_SEGS_S, 128 * SEG_ELEMS], F8,
                          kind="ExternalInput").ap()
    w4f_d = nc.dram_tensor("w4f_blob", [RES_SEGS[("fwd", 4)], 128 * SEG_ELEMS],
                           F8, kind="ExternalInput").ap()
    w4b_d = nc.dram_tensor("w4b_blob", [RES_SEGS[("bwd", 4)], 128 * SEG_ELEMS],
                           F8, kind="ExternalInput").ap()
    wi_d = nc.dram_tensor("wi_blob", [N_SEGS_I, 128 * SEG_ELEMS_I], BF16,
                          kind="ExternalInput").ap()
    ident_d = nc.dram_tensor("ident", [128, 128], BF16, kind="ExternalInput").ap()
    ident32_d = nc.dram_tensor("ident32", [128, 128], F32, kind="ExternalInput").ap()
    noise_d = nc.dram_tensor("noise", [steps * ROWS, E_TOT], F8,
                             kind="ExternalInput").ap()
    bstr_d = nc.dram_tensor("bstream", [steps * ROWS, SIZES[1]], BF16,
                            kind="ExternalInput").ap()
    out_d = nc.dram_tensor("out", [ROWS, SIZES[4]], F32, kind="ExternalOutput").ap()

    with tile.TileContext(nc) as tc, contextlib.ExitStack() as st:
        # persistent pools first (released never); init pools are scoped below
        c.wpool = st.enter_context(tc.tile_pool(name="wstream", bufs=3))
        c.apool = st.enter_context(tc.tile_pool(name="acc", bufs=6, space="PSUM"))
        c.tpool = st.enter_context(tc.tile_pool(name="tr", bufs=2, space="PSUM"))
        sp = st.enter_context(tc.tile_pool(name="state", bufs=1))
        npool = st.enter_context(tc.tile_pool(name="nzpool", bufs=1))
        sc = st.enter_context(tc.tile_pool(name="scratch", bufs=2))
        tp = st.enter_context(tc.tile_pool(name="tTpool", bufs=1))

        x = sp.tile([128, W_TOT], F32)
        e = sp.tile([128, E_TOT], F8)       # holds 16*e (clamped +-224)
        ident = sp.tile([128, 128], BF16)
        c.ident = ident
        nc.sync.dma_start(ident[:], ident_d)
        eT = {lo: sp.tile([128, _cdiv(SIZES[lo], 128) * 128], F8, tag=f"eT{lo}", name=f"eT{lo}")
              for lo in (1, 2, 3)}          # holds (16*e)^T fp8

        def x_ap(li, n0=0, nw=None):
            nw = SIZES[li] if nw is None else nw
            o = int(XOFS[li]) + n0
            return x[:, o:o + nw]

        def e_ap(lo, n0=0, nw=None):
            nw = SIZES[lo] if nw is None else nw
            o = int(EOFS[lo]) + n0
            return e[:, o:o + nw]

        # dedicated tT buffers for layers 3/4 (pre-built a step ahead)
        tT3 = sp.tile([128, _cdiv(SIZES[3], 128) * 128], F8, tag="tT3d",
                      name="tT3d")
        tT4 = sp.tile([128, _cdiv(SIZES[4], 128) * 128], F8, tag="tT4d",
                      name="tT4d")

        def build_tT(li, dst=None):
            tT = tp.tile([128, 32 * 128], F8, tag="tT", name="tT") \
                if dst is None else dst
            if li == 4:  # zero block 7 first; rows 1000..1023 must be 0
                nc.vector.memset(tT[:, 896:1024], 0.0)
            for (s0, swd) in _chunks(SIZES[li], 1024):
                tb = sc.tile([128, 1024], BF16, tag="tcast", name="tcast", bufs=1)
                for (n0, nw) in _chunks(swd, CHUNK):
                    nc.scalar.activation(tb[:, n0:n0 + nw],
                                         x_ap(li, s0 + n0, nw),
                                         mybir.ActivationFunctionType.Tanh)
                _emit_transpose(c, tT[:, s0:s0 + _cdiv(swd, 128) * 128],
                                tb[:, :swd], swd)
            return tT

        # ---------------- init: x_li = x_{li-1} @ W_li  (f32r from bf16)
        def init_out(li, n0, nw, acc):
            nc.vector.tensor_copy(x_ap(li, n0, nw), acc[:, :nw])

        with contextlib.ExitStack() as ist:
            c.ipool = ist.enter_context(tc.tile_pool(name="iwstream", bufs=4))
            xpool = ist.enter_context(tc.tile_pool(name="xTinit", bufs=1))
            ident32 = xpool.tile([128, 128], F32, tag="id32", name="ident32")
            c.ident32 = ident32
            nc.sync.dma_start(ident32[:], ident32_d)
            init_ofs = [0]
            xT_prev = xpool.tile([128, 32 * 128], F32R, tag="xTinit", name="xTinit")
            nc.sync.dma_start(
                xT_prev[:, :SIZES[0]].rearrange("p (k f) -> p k f", f=128),
                obsT_d.rearrange("k (p f) -> p k f", p=128))
            for li in (1, 2, 3, 4):
                _emit_layer_mm_init(c, li, xT_prev, wi_d, init_ofs, init_out)
                if li < 4:
                    xT_prev = xpool.tile([128, 32 * 128], F32R, tag="xTinit",
                                         name="xTinit")
                    _emit_transpose(c, xT_prev[:, :SIZES[li]], x_ap(li), SIZES[li],
                                    clamp=False, pdt=F32)

        # resident layer-4 weight tiles (both orientations), loaded once.
        # Allocated AFTER the init pools release so the stack allocator
        # reuses their region (init peak and settle peak both fit).
        rpool = st.enter_context(tc.tile_pool(name="w4res", bufs=1))
        w4res = {}
        for (kind, li), blob_d in ((("fwd", 4), w4f_d), (("bwd", 4), w4b_d)):
            tiles = []
            for si, used in enumerate(RES_USED[(kind, li)]):
                t = rpool.tile([128, used], F8, tag=f"w4{kind}{si}",
                               name=f"w4{kind}{si}")
                nc.sync.dma_start(
                    t[:],
                    blob_d[si:si + 1]
                        .rearrange("b (p f) -> p (b f)", p=128)[:, :used])
                tiles.append(t)
            w4res[(kind, li)] = tiles
        # pin the first N_PIN stream segs (fwd3's head) resident too: they
        # are the first consumed after each loop barrier, and each pinned
        # seg saves 1MB/step of HBM traffic
        pin_tiles = []
        for si in range(N_PIN):
            t = rpool.tile([128, SEG_ELEMS], F8, tag=f"pin{si}", name=f"pin{si}")
            nc.sync.dma_start(
                t[:],
                ws_d[si:si + 1].rearrange("b (p f) -> p (b f)", p=128))
            pin_tiles.append(t)

        # ---------------- settling steps (merged per-layer fwd/bwd sweep)
        def step_body(nz_src, b_src):
            nzs = npool.tile([128, E_TOT], F8, tag="nzstep", name="nzstep")
            nc.scalar.dma_start(nzs[:], nz_src)
            bst = npool.tile([128, SIZES[1] // 2], BF16, tag="bstep",
                              name="bstep")
            nc.scalar.dma_start(bst[:], b_src[:, :SIZES[1] // 2])
            stream_ofs = [N_PIN]

            def fwd_out(li_, n0, nw, acc):
                _lo = li_ - 1
                o = int(EOFS[_lo]) + n0
                # acc = 16*pred ; e16 = 16*x - acc + nz16, clamped into fp8 e
                eb = sc.tile([128, CHUNK], BF16, tag="ebuf", name="ebuf", bufs=1)
                nc.vector.scalar_tensor_tensor(
                    eb[:, :nw], acc[:, :nw], -1.0, nzs[:, o:o + nw],
                    mybir.AluOpType.mult, mybir.AluOpType.add)
                nc.vector.scalar_tensor_tensor(
                    eb[:, :nw], x_ap(_lo, n0, nw), WS, eb[:, :nw],
                    mybir.AluOpType.mult, mybir.AluOpType.add)
                nc.vector.tensor_scalar(
                    e_ap(_lo, n0, nw), eb[:, :nw], 224.0, -224.0,
                    mybir.AluOpType.min, mybir.AluOpType.max)
                # incremental eT: transpose this chunk now (keeps the
                # fwd->bwd transition off the critical path)
                _emit_transpose(c, eT[_lo][:, n0:n0 + nw], eb[:, :nw], nw)

            # d_actv(x) = 1 - tanh(x)^2 precomputed per chunk BEFORE the
            # group's matmuls close, so on acc-close one DVE op frees the
            # PSUM bank (the next layer's accs rotate in without stalling
            # the PE behind a serial ACT chain).
            d2map = {}

            def bwd_pre(li, n0, nw):
                if li == 1 and n0 == SIZES[1] // 2:
                    # refill the half-size B buffer for g1's second group
                    # (grp0's reads are already emitted; WAR is tracked)
                    nc.scalar.dma_start(bst[:], b_src[:, SIZES[1] // 2:])
                d2 = sc.tile([128, CHUNK], BF16, tag="d2", name="d2", bufs=3)
                nc.scalar.activation(d2[:, :nw], x_ap(li, n0, nw),
                                     mybir.ActivationFunctionType.Tanh)
                nc.scalar.activation(d2[:, :nw], d2[:, :nw],
                                     mybir.ActivationFunctionType.Square)
                nc.vector.tensor_scalar(d2[:, :nw], d2[:, :nw], -1.0, 1.0,
                                        mybir.AluOpType.mult, mybir.AluOpType.add)
                d2map[(li, n0)] = d2

            def bwd_out(li, n0, nw, acc, sub_b=False):
                # acc = 256*g  (for g1: g*256 = B256 - acc, sub_b=True)
                d2 = d2map.pop((li, n0))
                gd = sc.tile([128, CHUNK], BF16, tag="gd", name="gd", bufs=1)
                if sub_b:
                    gb = sc.tile([128, CHUNK], BF16, tag="gb", name="gb")
                    nc.vector.scalar_tensor_tensor(
                        gb[:, :nw], acc[:, :nw], -1.0,
                        bst[:, n0 % (SIZES[1] // 2):n0 % (SIZES[1] // 2) + nw],
                        mybir.AluOpType.mult, mybir.AluOpType.add)
                    nc.vector.tensor_mul(gd[:, :nw], gb[:, :nw], d2[:, :nw])
                else:
                    nc.vector.tensor_mul(gd[:, :nw], acc[:, :nw], d2[:, :nw])
                if li < 4:  # e4 is identically zero in the reference
                    nc.vector.scalar_tensor_tensor(
                        gd[:, :nw], e_ap(li, n0, nw), -WS, gd[:, :nw],
                        mybir.AluOpType.mult, mybir.AluOpType.add)
                nc.vector.scalar_tensor_tensor(
                    x_ap(li, n0, nw), gd[:, :nw], GAMMA / (WS * WS),
                    x_ap(li, n0, nw),
                    mybir.AluOpType.mult, mybir.AluOpType.add)

            def g1_out(li, n0, nw, acc):
                bwd_out(li, n0, nw, acc, sub_b=True)

            # fwd3/fwd4 consume tT built LAST step (right after bwd3/bwd4
            # finalized x3/x4) so the PE rolls from g1 straight into fwd3
            # without an ACT-tanh wait (keeps the HAM clock warm).
            _emit_layer_mm(c, 3, "fwd", tT3, fwd_out, blob=ws_d, ofs=stream_ofs,
                           res_prefix=pin_tiles)
            _emit_layer_mm(c, 4, "fwd", tT4, fwd_out, res=w4res[("fwd", 4)])
            _emit_layer_mm(c, 4, "bwd", eT[3], bwd_out, res=w4res[("bwd", 4)],
                           pre_cb=bwd_pre)
            build_tT(4, tT4)        # for next step (x4 is final)
            _emit_layer_mm(c, 3, "bwd", eT[2], bwd_out, blob=ws_d,
                           ofs=stream_ofs, pre_cb=bwd_pre)
            build_tT(3, tT3)        # for next step (x3 is final)
            tT2 = build_tT(2)
            _emit_layer_mm(c, 2, "fwd", tT2, fwd_out, blob=ws_d, ofs=stream_ofs)
            tT1 = build_tT(1)       # early: tanh(x1-old) overlaps bwd2 stream
            _emit_layer_mm(c, 2, "bwd", eT[1], bwd_out, blob=ws_d,
                           ofs=stream_ofs, pre_cb=bwd_pre)
            # layer 1 via the pinned-x0 identity: g1 = B(s) - t1 G1
            _emit_layer_mm(c, 1, "g1", tT1, g1_out, blob=ws_d, ofs=stream_ofs,
                           pre_cb=bwd_pre)

        # prologue: tT3/tT4 from the init-pass x values
        build_tT(3, tT3)
        build_tT(4, tT4)

        def warmup():
            # post-barrier PE keep-warm: dummy transposes run while the
            # drained weight stream refills, so the HAM clock is at 2.4GHz
            # when the real matmuls arrive
            pt = c.tpool.tile([128, 1024], BF16, tag="tr")
            for j in range(8):
                nc.tensor.transpose(pt[:, (j % 8) * 128:(j % 8) * 128 + 128],
                                    ident[:, :], ident[:, :])

        unroll = UNROLL if steps % UNROLL == 0 else next(
            u for u in (5, 4, 2, 1) if steps % u == 0)
        with tc.For_i(0, steps // unroll, 1, hint_engines=(mybir.EngineType.PE, mybir.EngineType.DVE, mybir.EngineType.Activation, mybir.EngineType.SP), staggered_reset=True) as it:
            nz_base = noise_d[bass.ts(it, unroll * ROWS), :]
            b_base = bstr_d[bass.ts(it, unroll * ROWS), :]
            warmup()
            for u in range(unroll):
                step_body(nz_base[u * ROWS:(u + 1) * ROWS, :],
                          b_base[u * ROWS:(u + 1) * ROWS, :])

        nc.sync.dma_start(out_d, x_ap(4))
    nc.finalize()
    return nc


# ---------------------------------------------------------------- entry

_CACHE = {}


def _prep_maps(obs, Ws, bs, steps):
    noise, bstream = _noise_eff(steps, bs, obs, Ws[0])
    ws_blob = _pack_rows_fp8(Ws, STREAM_ENTRIES)
    w4f_blob = _pack_rows_fp8(Ws, [("fwd", 4)])
    w4b_blob = _pack_rows_fp8(Ws, [("bwd", 4)])
    wi_blob = _pack_init(Ws)
    ident = np.eye(128, dtype=NPBF16)
    ident32 = np.eye(128, dtype=np.float32)
    in_maps = []
    for cx in range(N_CORES):
        r0 = cx * ROWS
        obs_c = np.ascontiguousarray(obs[r0:r0 + ROWS])
        obsT_c = np.ascontiguousarray(
            obs_c.T.reshape(SIZES[0] // 128, 128 * ROWS))
        nz_c = np.ascontiguousarray(
            noise[:, r0:r0 + ROWS, :]).reshape(steps * ROWS, E_TOT)
        b_c = np.ascontiguousarray(
            bstream[:, r0:r0 + ROWS, :]).reshape(steps * ROWS, SIZES[1])
        in_maps.append({
            "obsT": obsT_c, "ws_blob": ws_blob,
            "w4f_blob": w4f_blob, "w4b_blob": w4b_blob, "wi_blob": wi_blob,
            "ident": ident, "ident32": ident32, "noise": nz_c,
            "bstream": b_c,
        })
    return in_maps


def kernel(**inputs):
    obs = np.asarray(inputs["obs"], np.float32)
    Ws = [np.asarray(inputs[f"W{i}"], np.float32) for i in range(1, 5)]
    bs = [np.asarray(inputs[f"b{i}"], np.float32) for i in range(1, 5)]
    steps = int(inputs["steps"])
    assert obs.shape == (BATCH, SIZES[0])

    if steps not in _CACHE:
        _CACHE[steps] = build(steps)
    nc = _CACHE[steps]

    in_maps = _prep_maps(obs, Ws, bs, steps)
    res = bass_utils.run_bass_kernel_spmd(
        nc, in_maps, core_ids=list(range(N_CORES)), trace=False)
    return np.concatenate(
        [res.results[cx]["out"] for cx in range(N_CORES)], 0).astype(np.float32)
